# revision 20
# baseline (speedup 1.0000x reference)
"""Trainium2 Bass kernel for a single causal attention head (v3).

Problem: x [8, 2048, 1024] f32, Wq/Wk/Wv [1024, 64] f32.
out[b] = softmax(causal(x[b] Wq (x[b] Wk)^T) / 8) @ (x[b] Wv)   -> [8, 2048, 64] f32

Sharding: data-parallel over batch. Each of the 8 NeuronCores runs the same
single-core program on its own batch element (no collectives).

Per-body dataflow (matmuls in bf16, fp32 PSUM accumulation):
  1. x loads in 16 single-s-tile SWDGE cast-DMAs (f32->bf16).  Weights load
     f32 via sync HWDGE + DVE cast so the Pool SWDGE queue stays on x.
  2. x^T via PE matmul-by-identity, per s-tile, psum copies alternate DVE/Act.
  3. Packed [Wq|Wk] stationary: one matmul per (q-chunk, d-chunk) produces
     Q^T and K^T together; V^T separate; V natural (+ones col) via PE.
  4. scoresT[k, q] exact-causal from column kt*128, exp on Act (1/8 folded
     in), diagonal-block trimask on DVE.
  5. attnT @ [V | ones] accumulated per 512-wide output quarter (walrus
     needs uniform element counts per psum accumulation group); quarters
     drain early: O^T -> xbar transpose -> reciprocal-normalize -> DMA out.
  6. Attention half 0 (kt 0-7, q 0:1024) is emitted between projection
     chunks 1 and 2 so it runs while load chunks 8-15 arrive.

Timing loop: bodies are emitted back-to-back inside one For_i iteration
(`unroll` per iteration).  Per-engine in-order streams make adjacent bodies
pipeline (body i+1's loads run during body i's attention); the For_i
all-engine barrier only hits every `unroll` bodies.  Constants (identity,
trimask, act table) are set up once per NEFF execution.
"""

import contextlib
import math
import sys

import numpy as np

if "/opt/trn_rl_repo" not in sys.path:
    sys.path.insert(0, "/opt/trn_rl_repo")

import concourse.bacc as bacc
import concourse.tile as tile
from concourse import mybir
from concourse.masks import make_identity

BATCH = 8
SEQ = 2048
D_EMBED = 1024
HEAD = 64
N_CORES = 8

F32 = mybir.dt.float32
BF16 = mybir.dt.bfloat16


def build_attention_nc(S=SEQ, D=D_EMBED, repeat=1, phase="full",
                       sp_w=512, fps_bufs=3, sp_bufs=3, nbody=1, unroll=8,
                       av_lag=2, atn_bufs=4, pool_copy=False):
    """Build the single-core Bass program for one batch element."""
    H = HEAD
    ST = S // 128          # s-tiles (16)
    DC = D // 128          # d-chunks (8)
    QW = 512               # q-chunk width
    HW_ = S // 2           # half width (1024)
    inv_sqrt_h = 1.0 / math.sqrt(H)

    nc = bacc.Bacc("TRN2", target_bir_lowering=False, debug=False)

    x_dram = nc.dram_tensor("x", [S, D], F32, kind="ExternalInput").ap()
    wq_dram = nc.dram_tensor("Wq", [D, H], F32, kind="ExternalInput").ap()
    wk_dram = nc.dram_tensor("Wk", [D, H], F32, kind="ExternalInput").ap()
    wv_dram = nc.dram_tensor("Wv", [D, H], F32, kind="ExternalInput").ap()
    out_dram = nc.dram_tensor("out", [S, H], F32, kind="ExternalOutput").ap()
    out_r = out_dram.rearrange("(t p) h -> p t h", p=128)

    if repeat > 1 and repeat % unroll:
        unroll = next(u for u in (8, 5, 4, 2, 1) if repeat % u == 0)

    with tile.TileContext(nc) as tc:
        with (
            tc.tile_pool(name="sb", bufs=1) as sb,
            tc.tile_pool(name="fps", bufs=fps_bufs, space="PSUM") as fps,
            tc.tile_pool(name="aps", bufs=1, space="PSUM") as aps,
            tc.tile_pool(name="atn", bufs=atn_bufs) as atn,
        ):
            # ---------------- persistent SBUF ----------------
            xt2 = sb.tile([128, ST * DC, 128], BF16)
            ident = sb.tile([128, 128], BF16)
            trimask = sb.tile([128, 128], BF16)
            wqk = sb.tile([128, DC, 128], BF16)   # [Wq | Wk] packed
            wv_sb = sb.tile([128, DC, H], BF16)
            wq_f = sb.tile([128, DC, H], F32)
            wk_f = sb.tile([128, DC, H], F32)
            wv_f = sb.tile([128, DC, H], F32)
            dum = sb.tile([128, 1], BF16)

            x_src = x_dram.rearrange("(a p) d -> p a d", p=128)

            # ------------- one-time constants -------------
            make_identity(nc, ident)
            # trimask[k_local, q_local] = 1.0 if q_local >= k_local else 0.0
            nc.gpsimd.memset(trimask, 1.0)
            nc.gpsimd.affine_select(
                out=trimask,
                in_=trimask,
                compare_op=mybir.AluOpType.is_ge,
                fill=0.0,
                base=0,
                pattern=[[1, 128]],
                channel_multiplier=-1,
            )
            # preload the Exp activation table while everything idles
            nc.scalar.activation(
                out=dum, in_=ident[:, 0:1],
                func=mybir.ActivationFunctionType.Exp,
            )

            B = {}

            def alloc_body_tiles():
                B["x_bf"] = sb.tile([128, ST, D], BF16, name="x_bf",
                                    tag="xbf", bufs=2)
                B["qk_a"] = sb.tile([128, S], BF16, name="qk_a",
                                    tag="qka", bufs=2)
                B["qk_b"] = sb.tile([128, S], BF16, name="qk_b",
                                    tag="qkb", bufs=2)
                B["vt"] = sb.tile([64, S], BF16, name="vt", tag="vt", bufs=2)
                B["vnat"] = sb.tile([128, ST, H + 1], BF16, name="vnat",
                                    tag="vnat", bufs=2)
                B["osb"] = sb.tile([80, S], BF16, name="osb",
                                   tag="osb", bufs=2)
                B["onat"] = sb.tile([128, ST, 80], BF16, name="onat",
                                    tag="onat", bufs=2)
                B["o_out"] = sb.tile([128, ST, H], F32, name="o_out",
                                     tag="oo", bufs=2)
                B["rcp"] = sb.tile([128, ST], F32, name="rcp",
                                   tag="rcp", bufs=2)
                nc.vector.memset(B["vnat"][:, :, H:H + 1], 1.0)
                nc.vector.memset(B["osb"][64:80, :], 0.0)

            # ---------------- frontend helpers ----------------
            def xt_rhs(j, qc):
                # [128, 4, 128]: x^T d-chunk j for q-chunk qc
                return xt2[:, qc * 4 * DC + j:(qc + 1) * 4 * DC:DC, :]

            def emit_xpose(si):
                for g in range(2):
                    xp = fps.tile([128, 512], F32, name="xp", tag="f")
                    for k in range(4):
                        j = g * 4 + k
                        nc.tensor.matmul(
                            xp[:, k * 128:(k + 1) * 128],
                            lhsT=B["x_bf"][:, si, j * 128:(j + 1) * 128],
                            rhs=ident,
                            start=True, stop=True,
                        )
                    dst = xt2[:, si * DC + g * 4:si * DC + g * 4 + 4, :]
                    if (si + g) % 2 == 0:
                        nc.vector.tensor_copy(dst, xp)
                    else:
                        nc.scalar.copy(dst, xp)

            def emit_proj(qc):
                qsl = slice(qc * QW, (qc + 1) * QW)
                pp = fps.tile([128, QW], F32, name="pp", tag="f")
                for j in range(DC):
                    nc.tensor.matmul(
                        pp, lhsT=wqk[:, j, :], rhs=xt_rhs(j, qc),
                        start=(j == 0), stop=(j == DC - 1),
                    )
                nc.vector.tensor_copy(B["qk_a"][:, qsl], pp)
                nc.sync.dma_start(out=B["qk_b"][0:64, qsl], in_=B["qk_a"][64:128, qsl])
                nc.sync.dma_start(out=B["qk_b"][64:128, qsl], in_=B["qk_a"][0:64, qsl])
                pv = fps.tile([128, QW], F32, name="pv", tag="f")
                for j in range(DC):
                    nc.tensor.matmul(
                        pv[0:64, :], lhsT=wv_sb[:, j, :], rhs=xt_rhs(j, qc),
                        start=(j == 0), stop=(j == DC - 1),
                    )
                nc.vector.tensor_copy(B["vt"][:, qsl], pv[0:64, :])

            def emit_vnat(qc):
                vp = fps.tile([128, 4, H], F32, name="vp", tag="f")
                for t in range(4):
                    st = qc * 4 + t
                    nc.tensor.matmul(
                        vp[:, t, :],
                        lhsT=B["vt"][:, st * 128:(st + 1) * 128],
                        rhs=ident[0:64, 0:64],
                        start=True, stop=True,
                    )
                nc.scalar.copy(B["vnat"][:, qc * 4:(qc + 1) * 4, 0:H], vp)

            # ---------------- attention helpers ----------------
            def scores_mm(kt, dst, c0, c1):
                col = slice(kt * 128, (kt + 1) * 128)
                if kt % 2 == 0:
                    nc.tensor.matmul(
                        dst, lhsT=B["qk_b"][0:64, col], rhs=B["qk_a"][0:64, c0:c1],
                        start=True, stop=True,
                    )
                else:
                    nc.tensor.matmul(
                        dst, lhsT=B["qk_a"][64:128, col], rhs=B["qk_b"][64:128, c0:c1],
                        start=True, stop=True,
                    )

            def finalize_q(q, opsum):
                # drain quarter q (columns [q*512, (q+1)*512)) of O^T
                q_lo = q * QW
                nc.vector.tensor_copy(B["osb"][0:H + 1, q_lo:q_lo + QW], opsum)
                nc.sync.dma_start(
                    out=B["onat"][:, q * 4:(q + 1) * 4, :],
                    in_=B["osb"][0:80, q_lo:q_lo + QW],
                    transpose=True,
                )
                for t in range(q * 4, (q + 1) * 4):
                    nc.vector.reciprocal(B["rcp"][:, t:t + 1], B["onat"][:, t, H:H + 1])
                    nc.vector.tensor_scalar_mul(
                        B["o_out"][:, t, :], B["onat"][:, t, 0:H], B["rcp"][:, t:t + 1]
                    )
                nc.sync.dma_start(
                    out=out_r[:, q * 4:(q + 1) * 4, :],
                    in_=B["o_out"][:, q * 4:(q + 1) * 4, :],
                )

            def attn_half(h, interleave=None, drain_early=False):
                h_lo, h_hi = h * HW_, (h + 1) * HW_
                n_kt = h_hi // 128
                # the two output quarters of this half, each its own uniform
                # [H+1, 512] accumulation group (walrus requires uniform
                # element counts within a psum accumulation group)
                ops = {}
                for q in (2 * h, 2 * h + 1):
                    ops[q] = aps.tile([H + 1, QW], F32, name="opsum",
                                      tag="o", bufs=2)

                def emit_attnV(kt, at):
                    for q in (2 * h, 2 * h + 1):
                        if kt // 4 > q:
                            continue
                        nc.tensor.matmul(
                            ops[q],
                            lhsT=B["vnat"][:, kt, :],
                            rhs=at[:, q * QW:(q + 1) * QW],
                            start=(kt == 0),
                            stop=(kt == 4 * q + 3),
                            skip_group_check=True,
                        )
                    # early drain: quarter 2h completes at kt = 8h + 3
                    if drain_early and kt == 8 * h + 3:
                        finalize_q(2 * h, ops[2 * h])

                pending = []
                for kt in range(n_kt):
                    lo = max(h_lo, kt * 128)
                    at = atn.tile([128, S], BF16, name="attn", tag="at")
                    c = lo
                    while c < h_hi:
                        ce = min(c + sp_w, h_hi)
                        sp = aps.tile([128, sp_w], F32, name="sp", tag="s",
                                      bufs=sp_bufs)
                        scores_mm(kt, sp[:, 0:ce - c], c, ce)
                        nc.scalar.activation(
                            out=at[:, c:ce], in_=sp[:, 0:ce - c],
                            func=mybir.ActivationFunctionType.Exp,
                            scale=inv_sqrt_h,
                        )
                        c = ce
                    if kt * 128 >= h_lo:  # diagonal block lives in this half
                        nc.vector.tensor_mul(
                            at[:, kt * 128:(kt + 1) * 128],
                            at[:, kt * 128:(kt + 1) * 128],
                            trimask,
                        )
                        if kt % 4:  # zero [dq*512, kt*128) below-diag cols
                            nc.vector.memset(
                                at[:, (kt // 4) * QW:kt * 128], 0.0
                            )
                    pending.append((kt, at))
                    if len(pending) > av_lag:
                        emit_attnV(*pending.pop(0))
                    if interleave is not None and kt == 2:
                        interleave()
                for p in pending:
                    emit_attnV(*p)
                if drain_early:
                    finalize_q(2 * h + 1, ops[2 * h + 1])
                return ops

            # ---------------- one body ----------------
            def emit_body():
                alloc_body_tiles()
                for c in range(ST):
                    nc.gpsimd.dma_start(out=B["x_bf"][:, c, :], in_=x_src[:, c, :])
                for wf, wd in ((wq_f, wq_dram), (wk_f, wk_dram),
                               (wv_f, wv_dram)):
                    nc.sync.dma_start(
                        out=wf, in_=wd.rearrange("(j p) h -> p j h", p=128)
                    )
                nc.vector.tensor_copy(wqk[:, :, 0:H], wq_f)
                nc.vector.tensor_copy(wqk[:, :, H:128], wk_f)
                nc.vector.tensor_copy(wv_sb, wv_f)

                if phase == "load":
                    for t in range(ST):
                        nc.vector.tensor_copy(
                            B["o_out"][:, t, :],
                            B["x_bf"].rearrange("p a b -> p (a b)")
                                [:, t * H:(t + 1) * H],
                        )
                    nc.sync.dma_start(out=out_r, in_=B["o_out"])
                    return

                for si in range(4):
                    emit_xpose(si)
                emit_proj(0)
                for si in range(4, 8):
                    emit_xpose(si)
                emit_proj(1)
                emit_vnat(0)

                ops0 = attn_half(0, interleave=lambda: emit_vnat(1))

                for si in range(8, 12):
                    emit_xpose(si)
                emit_proj(2)
                finalize_q(0, ops0[0])
                finalize_q(1, ops0[1])
                for si in range(12, 16):
                    emit_xpose(si)
                emit_proj(3)
                emit_vnat(2)

                attn_half(1, interleave=lambda: emit_vnat(3),
                          drain_early=True)

            if repeat > 1:
                with tc.For_i(0, repeat // unroll, 1):
                    for _ in range(unroll):
                        emit_body()
            else:
                for _ in range(nbody):
                    emit_body()

    nc.compile()
    return nc


_NC_CACHE = {}


def _get_nc(S=SEQ, D=D_EMBED):
    key = (S, D)
    if key not in _NC_CACHE:
        _NC_CACHE[key] = build_attention_nc(S, D)
    return _NC_CACHE[key]


def kernel(x, Wq, Wk, Wv):
    """Full-input entry point: x [8, 2048, 1024] f32 -> [8, 2048, 64] f32."""
    from concourse.bass_utils import run_bass_kernel_spmd

    x = np.asarray(x, dtype=np.float32)
    Wq = np.ascontiguousarray(np.asarray(Wq, dtype=np.float32))
    Wk = np.ascontiguousarray(np.asarray(Wk, dtype=np.float32))
    Wv = np.ascontiguousarray(np.asarray(Wv, dtype=np.float32))
    assert x.shape == (BATCH, SEQ, D_EMBED), x.shape

    nc = _get_nc()
    in_maps = [
        {"x": np.ascontiguousarray(x[b]), "Wq": Wq, "Wk": Wk, "Wv": Wv}
        for b in range(BATCH)
    ]
    res = run_bass_kernel_spmd(nc, in_maps, core_ids=list(range(N_CORES)))
    return np.stack([res.results[b]["out"] for b in range(BATCH)], axis=0)
